# revision 1
# baseline (speedup 1.0000x reference)
"""Trainium2 Bass kernel for BinaryDecoderV2 — v3 (2x4 sharding).

Same pipeline as v2.5 (bit-packed weights, 3-pass nibble unpack, diagonal
pack matmuls, fp8 DoubleRow mains) but sharded 2-way over batch x 4-way over
out_features: per-core latent halves to 8.4MB, weights bits 2.1MB, true_sum
2.1MB -> ~12.7MB HBM/core instead of 20.2MB. PE main work is unchanged
(same MACs/core); pack work doubles (weight slice is 2x wider) but stays
far under the DMA roofline.

Core c: out-shard c%4 (256 outputs), batch-shard c//4 (1024 rows).
"""

import numpy as np
import ml_dtypes

IN_FEATURES = 8192
OUT_FEATURES = 1024
N_BITS = 8
BATCH = 2048
N_CORES = 8
OSH = 4                        # out-feature shards
BSH = 2                        # batch shards
OPC = OUT_FEATURES // OSH      # 256 outputs per core
BC = BATCH // BSH              # 1024 batch rows per core
KP = 128
KT = IN_FEATURES // KP         # 64 k-subtiles
DKT = KT // 2                  # 32 DoubleRow k-tiles
NRND = 32                      # pack rounds (2 kt each)
MEGA = 4                       # unpack mega-rounds (16 kt each)
NCHUNK = 512
NCH = BC // NCHUNK             # 2 batch chunks per core
_LCH = [16, 16, 16, 8, 4, 4]
LCH_START = {}
_s = 0
for _i, _n in enumerate(_LCH):
    LCH_START[_s] = (_i, _n)
    _s += _n
assert _s == KT
POWERS = [1.0, 2.0, 4.0, 8.0, 16.0, 32.0, 64.0, -128.0]
C_PLANES = [-0.5, -8.0]   # lo nibble, hi (sign-flipped) nibble
SCALE = 2.0 ** N_BITS - 1.0

_CACHE: dict = {}


def _build():
    import concourse.bacc as bacc
    import concourse.mybir as mybir
    from concourse import tile

    f8e4 = mybir.dt.float8e4
    u8 = mybir.dt.uint8
    f32 = mybir.dt.float32
    Act = mybir.ActivationFunctionType
    Alu = mybir.AluOpType
    PM = mybir.MatmulPerfMode

    nc = bacc.Bacc("TRN2", target_bir_lowering=False, debug=False,
                   num_devices=N_CORES)

    latq = nc.dram_tensor("latq", [128, KT, BC], f8e4,
                          kind="ExternalInput")
    w8 = nc.dram_tensor("w8", [128, KT, OPC], u8, kind="ExternalInput")
    tq = nc.dram_tensor("tq", [128, 2, N_BITS, BC], f8e4,
                        kind="ExternalInput")
    dg = nc.dram_tensor("dg", [128, N_BITS, 128], f8e4,
                        kind="ExternalInput")
    cp = nc.dram_tensor("cp", [128, 2, 128], f8e4, kind="ExternalInput")
    partials = nc.dram_tensor("partials", [128, 4], f32,
                              kind="ExternalOutput")

    with tile.TileContext(nc) as tc:
        with (
            tc.tile_pool(name="w8p", bufs=1) as w8_pool,
            tc.tile_pool(name="tsp", bufs=1) as tsp_pool,
            tc.tile_pool(name="cst", bufs=1) as cst_pool,
            tc.tile_pool(name="tp", bufs=2) as t_pool,
            tc.tile_pool(name="iw", bufs=1) as iw_pool,
            tc.tile_pool(name="lat", bufs=1) as lat_pool,
            tc.tile_pool(name="loss", bufs=1) as loss_pool,
            tc.tile_pool(name="ps", bufs=1, space="PSUM") as psum_pool,
            tc.tile_pool(name="pk", bufs=2, space="PSUM") as pk_pool,
        ):
            dgt = cst_pool.tile([128, N_BITS, 128], f8e4, name="dgt",
                                tag="dgt")
            nc.sync.dma_start(dgt[:], dg[:])
            cpt = cst_pool.tile([128, 2, 128], f8e4, name="cpt", tag="cpt")
            nc.sync.dma_start(cpt[:], cp[:])

            w8t = w8_pool.tile([128, KT, OPC], u8)
            tp = tsp_pool.tile([128, 2, N_BITS, BC], f8e4)
            nc.sync.dma_start(w8t[:, 0:16, :], w8[:, 0:16, :])
            nc.sync.dma_start(tp[:, 0, :, :], tq[:, 0, :, :])
            nc.sync.dma_start(w8t[:, 16:32, :], w8[:, 16:32, :])
            nc.sync.dma_start(tp[:, 1, :, :], tq[:, 1, :, :])
            nc.sync.dma_start(w8t[:, 32:48, :], w8[:, 32:48, :])
            nc.sync.dma_start(w8t[:, 48:64, :], w8[:, 48:64, :])

            # ---- all latent chunk DMAs issued up-front: dedicated
            # tiles -> no pool-reuse semaphores on the sync queue, the
            # striped DMA queue streams continuously ----
            lts = {}
            for kt0, (q, n) in LCH_START.items():
                lt = lat_pool.tile([128, n, BC], f8e4, name=f"lt{q}",
                                   tag=f"lat{q}")
                nc.sync.dma_start(lt[:], latq[:, kt0:kt0 + n, :])
                lts[kt0] = (lt, kt0)

            # ---- int_sum into 4 psums: index = oh*2 + ch ----
            psums = [psum_pool.tile([128, NCHUNK], f32, name=f"ps{i}",
                                    tag=f"ps{i}") for i in range(4)]
            for oh in range(2):
                for bp in range(4):
                    for ch in range(NCH):
                        nc.tensor.matmul(
                            psums[oh * NCH + ch][:],
                            dgt[:, 2 * bp:2 * bp + 2, :],
                            tp[:, oh, 2 * bp:2 * bp + 2,
                               ch * NCHUNK:(ch + 1) * NCHUNK],
                            start=(bp == 0), stop=False,
                            perf_mode=PM.DoubleRow)

            # ---- weight pipeline + main matmul stream ----
            iwts = [iw_pool.tile([128, 2, OPC], f8e4, name=f"iw{r}",
                                 tag=f"iw{r}") for r in range(NRND)]
            out_t = loss_pool.tile([128, 4], f32, name="out_t",
                                   tag="out_t")

            tts = {}

            def pack_round(r):
                # unpack 16 kt worth: 2 nibble planes [128, 2, 16*OPC]
                # (host pre-XORs 0x80, so hi' = x>>4 and the -128 constant
                # folds into the cast bias)
                mr, rr = divmod(r, 8)
                if rr == 0:
                    tt = t_pool.tile([128, 2, 16 * OPC], f8e4,
                                     name=f"tt{mr}", tag="tt")
                    w8s = w8t[:, 16 * mr:16 * (mr + 1), :]
                    nc.vector.tensor_scalar(tt[:, 0, :].bitcast(u8), w8s,
                                            15, None, Alu.bitwise_and)
                    nc.vector.tensor_scalar(tt[:, 1, :].bitcast(u8), w8s,
                                            4, None,
                                            Alu.logical_shift_right)
                    tts[mr] = tt
                tt = tts[mr]
                pkb = pk_pool.tile([128, NCHUNK], f32, name=f"pk{r}",
                                   tag="pk")
                nc.tensor.matmul(
                    pkb[:], cpt[:, 0:2, :],
                    tt[:, 0:2, rr * NCHUNK:(rr + 1) * NCHUNK],
                    start=True, stop=True, perf_mode=PM.DoubleRow)
                # intw = 1024*(-(n_lo+16*n_hi')/1024) + 128 = -int_w
                nc.scalar.activation(iwts[r][:], pkb[:], Act.Copy,
                                     scale=1024.0, bias=128.0)

            # pack/cast pipelined 2 rounds ahead of the main matmuls
            pack_round(0)
            pack_round(1)
            for r in range(NRND):
                dkt = r
                kt0 = 2 * dkt
                cur = lts[max(s for s in lts if s <= kt0)]
                lt, base = cur
                a = kt0 - base
                last = (dkt == DKT - 1)
                for oh in range(2):
                    lhsT = iwts[r][:, :, oh * 128:(oh + 1) * 128]
                    for ch in range(NCH):
                        nc.tensor.matmul(
                            psums[oh * NCH + ch][:], lhsT,
                            lt[:, a:a + 2,
                               ch * NCHUNK:(ch + 1) * NCHUNK],
                            start=False, stop=last,
                            perf_mode=PM.DoubleRow)
                        if last:
                            i4 = oh * NCH + ch
                            d2 = loss_pool.tile(
                                [128, NCHUNK], f32, name=f"d2_{i4}",
                                tag=f"d2_{i4}")
                            nc.scalar.activation(
                                d2[:], psums[i4][:], Act.Square,
                                accum_out=out_t[:, i4:i4 + 1])
                if r + 2 < NRND:
                    pack_round(r + 2)

            nc.sync.dma_start(partials[:], out_t[:])

    nc.compile()
    return nc


def _get_nc():
    if "nc" not in _CACHE:
        _CACHE["nc"] = _build()
    return _CACHE["nc"]


def make_in_maps(latent: np.ndarray, true_sum: np.ndarray,
                 weight: np.ndarray) -> list:
    f8 = ml_dtypes.float8_e4m3fn

    # latq per batch shard: latq[p, kt, n] = latent[sb*BC + n, kt*128 + p]
    lat8 = latent.astype(f8)
    latqs = []
    for sb in range(BSH):
        ls = lat8[sb * BC:(sb + 1) * BC, :]
        latqs.append(np.ascontiguousarray(
            ls.T.reshape(KT, KP, BC).transpose(1, 0, 2)))

    bits = (weight > 0).astype(np.uint8).reshape(IN_FEATURES,
                                                 OUT_FEATURES, N_BITS)
    shifts = (1 << np.arange(N_BITS, dtype=np.uint16))
    bytes_ko = ((bits.astype(np.uint16) * shifts).sum(-1)
                .astype(np.uint8) ^ 0x80)   # flip sign bit: -128 -> cast bias
    w8s = []
    for so in range(OSH):
        wcol = bytes_ko[:, so * OPC:(so + 1) * OPC]
        w8s.append(np.ascontiguousarray(
            wcol.reshape(KT, KP, OPC).transpose(1, 0, 2)))

    dg = np.zeros((128, N_BITS, 128), dtype=np.float32)
    for b in range(N_BITS):
        np.fill_diagonal(dg[:, b, :], POWERS[b])
    dg8 = dg.astype(f8)
    cpm = np.zeros((128, 2, 128), dtype=np.float32)
    for j in range(2):
        np.fill_diagonal(cpm[:, j, :], C_PLANES[j])
    cp8 = cpm.astype(f8)

    ts8 = true_sum.astype(f8)
    in_maps = []
    for c in range(N_CORES):
        so, sb = c % OSH, c // OSH
        # tq[o128, oh, b, n] = true_sum[sb*BC+n, (so*256 + oh*128 + o128)*8 + b]
        T = ts8[sb * BC:(sb + 1) * BC,
                so * OPC * N_BITS:(so + 1) * OPC * N_BITS]
        t5 = T.reshape(BC, 2, 128, N_BITS)       # [n, oh, o128, b]
        tql = np.ascontiguousarray(t5.transpose(2, 1, 3, 0))
        in_maps.append({"latq": latqs[sb], "w8": w8s[so], "tq": tql,
                        "dg": dg8, "cp": cp8})
    return in_maps


def kernel(latent: np.ndarray, true_sum: np.ndarray,
           weight: np.ndarray) -> np.ndarray:
    from concourse.bass_utils import run_bass_kernel_spmd

    nc = _get_nc()
    in_maps = make_in_maps(latent, true_sum, weight)
    res = run_bass_kernel_spmd(nc, in_maps, list(range(N_CORES)))

    total = 0.0
    for c in range(N_CORES):
        total += float(res.results[c]["partials"].astype(np.float64).sum())
    loss = total / (BATCH * OUT_FEATURES) / (SCALE * SCALE)
    return np.array(loss, dtype=np.float32)



# revision 5
# speedup vs baseline: 1.4064x; 1.4064x over previous
"""Trainium2 Bass kernel for BinaryDecoderV2 — v4 (host-precomputed f8 weights).

Key insight over v3: the device-side weight unpack produced f8e4m3 weights
anyway, so the host can ship f8(-int_w) directly at the SAME byte count
(1 B/weight) — no vector nibble ops, no pack matmuls, no scalar casts.
Likewise int_sum is a pure function of the input true_sum, so the host
precomputes it and ships bf16 [batch, out] (8x smaller than f8 bit-planes).

Sharding: 4-way over batch x 2-way over out_features (per-core HBM:
latent 4.2MB + weights 4.2MB + int_sum 0.5MB ~= 8.9MB, vs 12.7MB in v3).

Device per core: stream k-ordered (weight, latent) tile groups; 128
DoubleRow fp8 matmuls accumulate -pred into 4 PSUM banks [128 out, 512
batch]; int_sum is injected mid-stream via 4 identity-lhsT bf16 matmuls
(PSUM then holds int_sum - pred = -255*diff); 4 vector tensor_tensor_reduce
ops square+sum each bank; host sums the 8x[128,4] partials.

Core c: out-shard c%2 (512 outputs), batch-shard c//2 (512 rows).
"""

import numpy as np
import ml_dtypes

IN_FEATURES = 8192
OUT_FEATURES = 1024
N_BITS = 8
BATCH = 2048
N_CORES = 8
OSH = 2                        # out-feature shards
BSH = 4                        # batch shards
OPC = OUT_FEATURES // OSH      # 512 outputs per core
BC = BATCH // BSH              # 512 batch rows per core
KP = 128
KT = IN_FEATURES // KP         # 64 k-subtiles
DKT = KT // 2                  # 32 DoubleRow k-tile rounds
NOH = OPC // 128               # 4 psum tiles (128 outputs each)
# k-group sizes (in kt units) for DMA chunking: small at the edges so the
# first matmul starts early and the last rounds wait on small transfers.
KGROUPS = [2, 2, 4, 8, 8, 8, 8, 8, 8, 4, 2, 2]
assert sum(KGROUPS) == KT
INJECT_ROUND = 4               # int_sum inject after this many DR rounds
SCALE = 2.0 ** N_BITS - 1.0
POWERS = [1.0, 2.0, 4.0, 8.0, 16.0, 32.0, 64.0, -128.0]

_CACHE: dict = {}


def _build():
    import concourse.bacc as bacc
    import concourse.mybir as mybir
    from concourse import tile

    f8e4 = mybir.dt.float8e4
    bf16 = mybir.dt.bfloat16
    f32 = mybir.dt.float32
    Act = mybir.ActivationFunctionType
    PM = mybir.MatmulPerfMode

    nc = bacc.Bacc("TRN2", target_bir_lowering=False, debug=False,
                   num_devices=N_CORES)

    latq = nc.dram_tensor("latq", [128, KT, BC], f8e4, kind="ExternalInput")
    w8f = nc.dram_tensor("w8f", [128, KT, OPC], f8e4, kind="ExternalInput")
    ints = nc.dram_tensor("ints", [128, NOH, BC], bf16, kind="ExternalInput")
    dg = nc.dram_tensor("dg", [128, 128], bf16, kind="ExternalInput")
    partials = nc.dram_tensor("partials", [128, NOH], f32,
                              kind="ExternalOutput")

    with tile.TileContext(nc) as tc:
        with (
            tc.tile_pool(name="wp", bufs=1) as w_pool,
            tc.tile_pool(name="lp", bufs=1) as l_pool,
            tc.tile_pool(name="cst", bufs=1) as cst_pool,
            tc.tile_pool(name="out", bufs=1) as out_pool,
            tc.tile_pool(name="ps", bufs=1, space="PSUM") as psum_pool,
        ):
            # ---- all input DMAs issued up-front on two engines, dedicated
            # tiles (no pool-reuse semaphores); k-interleaved group order so
            # round r's weight+latent tiles land together ----
            dgt = cst_pool.tile([128, 128], bf16, name="dgt", tag="dgt")
            intt = cst_pool.tile([128, NOH, BC], bf16, name="intt", tag="intt")
            wts, lts = [], []
            kt0 = 0
            for gi, n in enumerate(KGROUPS):
                wt = w_pool.tile([128, n, OPC], f8e4, name=f"w{gi}",
                                 tag=f"w{gi}")
                lt = l_pool.tile([128, n, BC], f8e4, name=f"l{gi}",
                                 tag=f"l{gi}")
                nc.sync.dma_start(wt[:], w8f[:, kt0:kt0 + n, :])
                nc.gpsimd.dma_start(lt[:], latq[:, kt0:kt0 + n, :])
                if gi == 0:
                    nc.sync.dma_start(dgt[:], dg[:])
                    nc.gpsimd.dma_start(intt[:], ints[:])
                wts.append((wt, kt0))
                lts.append((lt, kt0))
                kt0 += n

            psums = [psum_pool.tile([128, BC], f32, name=f"ps{i}",
                                    tag=f"ps{i}") for i in range(NOH)]
            out_t = out_pool.tile([128, NOH], f32, name="out_t", tag="out_t")
            sqs = [out_pool.tile([128, BC], f32, name=f"sq{i}", tag=f"sq{i}")
                   for i in range(NOH)]

            # ---- main matmul stream: psum[oh] = int_sum - pred ----
            gi = 0
            for r in range(DKT):
                kt = 2 * r
                wt, wbase = wts[gi]
                lt, lbase = lts[gi]
                if kt - wbase >= KGROUPS[gi] :
                    gi += 1
                    wt, wbase = wts[gi]
                    lt, lbase = lts[gi]
                a = kt - wbase
                last = (r == DKT - 1)
                for oh in range(NOH):
                    nc.tensor.matmul(
                        psums[oh][:],
                        wt[:, a:a + 2, oh * 128:(oh + 1) * 128],
                        lt[:, a:a + 2, :],
                        start=(r == 0), stop=last,
                        perf_mode=PM.DoubleRow)
                    if last:
                        nc.scalar.activation(
                            sqs[oh][:], psums[oh][:], Act.Square,
                            accum_out=out_t[:, oh:oh + 1])
                if r == INJECT_ROUND:
                    for oh in range(NOH):
                        nc.tensor.matmul(
                            psums[oh][:], dgt[:],
                            intt[:, oh, :],
                            start=False, stop=False)

            nc.sync.dma_start(partials[:], out_t[:])

    nc.compile()
    return nc


def _get_nc():
    if "nc" not in _CACHE:
        _CACHE["nc"] = _build()
    return _CACHE["nc"]


def make_in_maps(latent: np.ndarray, true_sum: np.ndarray,
                 weight: np.ndarray) -> list:
    f8 = ml_dtypes.float8_e4m3fn
    bf = ml_dtypes.bfloat16

    # latq per batch shard: latq[p, kt, n] = latent[sb*BC + n, kt*128 + p]
    lat8 = latent.astype(f8)
    latqs = []
    for sb in range(BSH):
        ls = lat8[sb * BC:(sb + 1) * BC, :]
        latqs.append(np.ascontiguousarray(
            ls.T.reshape(KT, KP, BC).transpose(1, 0, 2)))

    # int weights from sign bits; ship f8(-int_w) directly
    bits = (weight > 0).reshape(IN_FEATURES, OUT_FEATURES, N_BITS)
    pw = np.asarray(POWERS, dtype=np.float32)
    int_w = bits.astype(np.float32) @ pw          # [in, out]
    w8_full = (-int_w).astype(f8)
    w8fs = []
    for so in range(OSH):
        wcol = w8_full[:, so * OPC:(so + 1) * OPC]
        w8fs.append(np.ascontiguousarray(
            wcol.reshape(KT, KP, OPC).transpose(1, 0, 2)))

    # int_sum precomputed exactly, shipped bf16
    int_sum = (true_sum.reshape(BATCH, OUT_FEATURES, N_BITS)
               .astype(np.float32) @ pw)          # [batch, out]
    ints_bf = int_sum.astype(bf)

    dgm = np.eye(128, dtype=np.float32).astype(bf)

    in_maps = []
    for c in range(N_CORES):
        so, sb = c % OSH, c // OSH
        # ints[o128, oh, n] = int_sum[sb*BC+n, so*OPC + oh*128 + o128]
        S = ints_bf[sb * BC:(sb + 1) * BC, so * OPC:(so + 1) * OPC]
        ic = np.ascontiguousarray(
            S.reshape(BC, NOH, 128).transpose(2, 1, 0))
        in_maps.append({"latq": latqs[sb], "w8f": w8fs[so], "ints": ic,
                        "dg": dgm})
    return in_maps


def kernel(latent: np.ndarray, true_sum: np.ndarray,
           weight: np.ndarray) -> np.ndarray:
    from concourse.bass_utils import run_bass_kernel_spmd

    nc = _get_nc()
    in_maps = make_in_maps(latent, true_sum, weight)
    res = run_bass_kernel_spmd(nc, in_maps, list(range(N_CORES)))

    total = 0.0
    for c in range(N_CORES):
        total += float(res.results[c]["partials"].astype(np.float64).sum())
    loss = total / (BATCH * OUT_FEATURES) / (SCALE * SCALE)
    return np.array(loss, dtype=np.float32)
